# revision 6
# baseline (speedup 1.0000x reference)
"""Trainium2 Bass kernel for AdjacencyConv2d (GNN message passing).

Computation (mask is handled host-side, matching the reference):
    gathered = in_feats[adj_ids]                  # [N, 9, 64]
    conv_out = gathered.reshape(N, 576) @ W.T + b # [N, 64]
    out = scatter-by-mask-rank(conv_out)

Sharding: rows (nodes) split evenly across 8 NeuronCores; in_feats is
replicated to every core's HBM; conv weight/bias replicated.

Per-core device pipeline, per block of 512 nodes (4 sub-blocks of 128):
  1. one indirect DMA gathers all 512*9 neighbor rows (256 B each) into an
     SBUF tile G[128 part = node-in-subblock, 4*9*64 free]
  2. PE transposes 128-col chunks of G into PSUM -> [feat, node] layout
  3. DVE/ACT copy PSUM->SBUF
  4. 5 accumulated matmuls (float32r, K=128/64) compute out[64 o, 512 n]
  5. ACT adds bias while evacuating PSUM, DMA writes out [o, n]-major
Host transposes the per-core [64, per_core] results back to [n, o].
"""

import numpy as np

N_FEATS = 400000
IN_CH = 64
OUT_CH = 64
KK = 9
N_CORES = 8
BLOCK = 512            # nodes per block
SUB = 128              # nodes per sub-block (partition dim)
SPB = BLOCK // SUB     # sub-blocks per block
JPB = SPB * KK         # gather indices per partition per block
FW = KK * IN_CH        # 576 flattened features per node
FCH = (128, 128, 128, 128, 64)   # contraction chunks covering 576
IDX_BATCH = 7          # blocks of adj indices per index-load DMA

_NC_CACHE = {}


def _build(nblocks, n_feats, n_cores):
    import concourse.bacc as bacc
    import concourse.bass as bass
    import concourse.mybir as mybir
    import concourse.tile as tile
    from concourse.masks import make_identity

    dt = mybir.dt
    nc = bacc.Bacc(
        "TRN2",
        target_bir_lowering=False,
        debug=False,
        num_devices=n_cores,
        dynamic_dma_scratch_size=98304,
    )
    feats = nc.dram_tensor("feats", [n_feats, IN_CH], dt.float32, kind="ExternalInput")
    adjt = nc.dram_tensor("adjt", [SUB, nblocks * JPB], dt.int32, kind="ExternalInput")
    wt = nc.dram_tensor("wt", [128, 5 * OUT_CH], dt.float32, kind="ExternalInput")
    biasw = nc.dram_tensor("biasw", [OUT_CH, 1], dt.float32, kind="ExternalInput")
    outp = nc.dram_tensor(
        "outp", [OUT_CH, nblocks * BLOCK], dt.float32, kind="ExternalOutput"
    )

    f32r = dt.float32r
    with tile.TileContext(nc) as tc:
        with (
            tc.tile_pool(name="const", bufs=1) as constp,
            tc.tile_pool(name="idx", bufs=2) as idxp,
            tc.tile_pool(name="gath", bufs=8) as gathp,
            tc.tile_pool(name="tsb", bufs=8) as tsbp,
            tc.tile_pool(name="osb", bufs=3) as osbp,
            tc.tile_pool(name="tps", bufs=4, space="PSUM") as tpsp,
            tc.tile_pool(name="ops", bufs=3, space="PSUM") as opsp,
        ):
            wt_f32 = constp.tile([128, 5 * OUT_CH], dt.float32)
            nc.sync.dma_start(out=wt_f32[:], in_=wt[:])
            wt_sb = constp.tile([128, 5 * OUT_CH], f32r)
            nc.vector.tensor_copy(out=wt_sb[:], in_=wt_f32[:])
            bias_sb = constp.tile([OUT_CH, 1], dt.float32)
            nc.sync.dma_start(out=bias_sb[:], in_=biasw[:])
            ident = constp.tile([128, 128], dt.float32)
            make_identity(nc, ident[:])

            idx_sb = None
            for b in range(nblocks):
                bi = b % IDX_BATCH
                if bi == 0:
                    nb = min(IDX_BATCH, nblocks - b)
                    idx_sb = idxp.tile([SUB, IDX_BATCH * JPB], dt.int32)
                    nc.sync.dma_start(
                        out=idx_sb[:, : nb * JPB],
                        in_=adjt[:, b * JPB : (b + nb) * JPB],
                    )
                gath = gathp.tile([SUB, SPB * FW], dt.float32)
                # HW contract: one index per partition per indirect DMA, each
                # gathering one contiguous 64-elem row to its partition.
                for j in range(JPB):
                    nc.gpsimd.indirect_dma_start(
                        out=gath[:].rearrange("p (j c) -> p j c", c=IN_CH)[:, j],
                        out_offset=None,
                        in_=feats[:],
                        in_offset=bass.IndirectOffsetOnAxis(
                            ap=idx_sb[:, bi * JPB + j : bi * JPB + j + 1], axis=0
                        ),
                    )
                out_ps = opsp.tile([OUT_CH, BLOCK], dt.float32, space="PSUM")
                for q in range(5):
                    kq = FCH[q]
                    t_ps = tpsp.tile([128, BLOCK], dt.float32, space="PSUM")
                    for sb in range(SPB):
                        src = gath[:, sb * FW + q * 128 : sb * FW + q * 128 + kq]
                        nc.tensor.transpose(
                            out=t_ps[:kq, sb * SUB : (sb + 1) * SUB],
                            in_=src,
                            identity=ident[:],
                        )
                    t_sb = tsbp.tile([128, BLOCK], f32r)
                    nc.any.tensor_copy(out=t_sb[:kq, :], in_=t_ps[:kq, :])
                    nc.tensor.matmul(
                        out=out_ps[:],
                        lhsT=wt_sb[:kq, q * OUT_CH : (q + 1) * OUT_CH],
                        rhs=t_sb[:kq, :],
                        start=(q == 0),
                        stop=(q == 4),
                    )
                out_sb = osbp.tile([OUT_CH, BLOCK], dt.float32)
                nc.scalar.activation(
                    out_sb[:],
                    out_ps[:],
                    mybir.ActivationFunctionType.Identity,
                    bias=bias_sb[:],
                )
                nc.sync.dma_start(
                    out=outp[:, b * BLOCK : (b + 1) * BLOCK], in_=out_sb[:]
                )
    nc.compile()
    return nc


def _get_nc(nblocks, n_feats, n_cores):
    key = (nblocks, n_feats, n_cores)
    if key not in _NC_CACHE:
        _NC_CACHE[key] = _build(nblocks, n_feats, n_cores)
    return _NC_CACHE[key]


def _prep_adjt(adj, per_core, nblocks):
    """adj [n, KK] int32 -> per-core [SUB, nblocks*JPB] index layout.

    adjt[c][p, b*JPB + sb*KK + k] = adj[c*per_core + b*BLOCK + sb*SUB + p, k]
    (zero padded past the real rows).
    """
    n = adj.shape[0]
    pad = np.zeros((N_CORES, nblocks * BLOCK, KK), np.int32)
    for c in range(N_CORES):
        lo = c * per_core
        cnt = max(0, min(per_core, n - lo))
        if cnt:
            pad[c, :cnt] = adj[lo : lo + cnt]
    # [c, nblocks, SPB, SUB, KK] -> [c, SUB, nblocks, SPB, KK]
    arr = pad.reshape(N_CORES, nblocks, SPB, SUB, KK)
    arr = np.ascontiguousarray(arr.transpose(0, 3, 1, 2, 4))
    return arr.reshape(N_CORES, SUB, nblocks * JPB)


def _prep_wt(W):
    """W [64, 576] -> [128, 5*64] chunked-transposed stationary layout."""
    WT = np.ascontiguousarray(W.T.astype(np.float32))  # [576, 64]
    wt = np.zeros((128, 5 * OUT_CH), np.float32)
    for q in range(5):
        kq = FCH[q]
        wt[:kq, q * OUT_CH : (q + 1) * OUT_CH] = WT[q * 128 : q * 128 + kq]
    return wt


def kernel(in_feats, mask, adj_ids, conv_weight, conv_bias):
    import sys

    if "/opt/trn_rl_repo" not in sys.path:
        sys.path.insert(0, "/opt/trn_rl_repo")
    from concourse.bass_utils import run_bass_kernel_spmd

    in_feats = np.ascontiguousarray(np.asarray(in_feats, dtype=np.float32))
    mask_np = np.asarray(mask).astype(bool)
    adj = np.asarray(adj_ids).astype(np.int32)
    W = np.asarray(conv_weight, dtype=np.float32)
    bvec = np.asarray(conv_bias, dtype=np.float32)

    n = adj.shape[0]
    n_feats = in_feats.shape[0]
    per_core = (n + N_CORES - 1) // N_CORES
    nblocks = (per_core + BLOCK - 1) // BLOCK

    adjt = _prep_adjt(adj, per_core, nblocks)
    wt_np = _prep_wt(W)
    bias_np = np.ascontiguousarray(bvec.reshape(OUT_CH, 1))

    nc = _get_nc(nblocks, n_feats, N_CORES)
    in_maps = [
        {
            "feats": in_feats,
            "adjt": np.ascontiguousarray(adjt[c]),
            "wt": wt_np,
            "biasw": bias_np,
        }
        for c in range(N_CORES)
    ]
    res = run_bass_kernel_spmd(nc, in_maps, core_ids=list(range(N_CORES)))
    globals()["_LAST_RESULTS"] = res

    conv = np.empty((n, OUT_CH), np.float32)
    for c in range(N_CORES):
        lo = c * per_core
        cnt = max(0, min(per_core, n - lo))
        if cnt:
            conv[lo : lo + cnt] = res.results[c]["outp"][:, :cnt].T
    if mask_np.all():
        return conv
    pos = np.cumsum(mask_np.astype(np.int32)) - 1
    pos = np.clip(pos, 0, n - 1)
    return np.where(mask_np[:, None], conv[pos], np.float32(0))



# revision 7
# speedup vs baseline: 1.0006x; 1.0006x over previous
"""Trainium2 Bass kernel for AdjacencyConv2d (GNN message passing).

Computation (mask is handled host-side, matching the reference):
    gathered = in_feats[adj_ids]                  # [N, 9, 64]
    conv_out = gathered.reshape(N, 576) @ W.T + b # [N, 64]
    out = scatter-by-mask-rank(conv_out)

Sharding: rows (nodes) split evenly across 8 NeuronCores; in_feats is
replicated to every core's HBM; conv weight/bias replicated.

Per-core device pipeline, per block of 512 nodes (4 sub-blocks of 128):
  1. one indirect DMA gathers all 512*9 neighbor rows (256 B each) into an
     SBUF tile G[128 part = node-in-subblock, 4*9*64 free]
  2. PE transposes 128-col chunks of G into PSUM -> [feat, node] layout
  3. DVE/ACT copy PSUM->SBUF
  4. 5 accumulated matmuls (float32r, K=128/64) compute out[64 o, 512 n]
  5. ACT adds bias while evacuating PSUM, DMA writes out [o, n]-major
Host transposes the per-core [64, per_core] results back to [n, o].
"""

import numpy as np

N_FEATS = 400000
IN_CH = 64
OUT_CH = 64
KK = 9
N_CORES = 8
BLOCK = 512            # nodes per block
SUB = 128              # nodes per sub-block (partition dim)
SPB = BLOCK // SUB     # sub-blocks per block
JPB = SPB * KK         # gather indices per partition per block
FW = KK * IN_CH        # 576 flattened features per node
FCH = (128, 128, 128, 128, 64)   # contraction chunks covering 576
IDX_BATCH = 14         # blocks of adj indices per index-load DMA

_NC_CACHE = {}


def _build(nblocks, n_feats, n_cores):
    import concourse.bacc as bacc
    import concourse.bass as bass
    import concourse.mybir as mybir
    import concourse.tile as tile
    from concourse.masks import make_identity

    dt = mybir.dt
    nc = bacc.Bacc(
        "TRN2",
        target_bir_lowering=False,
        debug=False,
        num_devices=n_cores,
        dynamic_dma_scratch_size=98304,
    )
    feats = nc.dram_tensor("feats", [n_feats, IN_CH], dt.float32, kind="ExternalInput")
    adjt = nc.dram_tensor("adjt", [SUB, nblocks * JPB], dt.int32, kind="ExternalInput")
    wt = nc.dram_tensor("wt", [128, 5 * OUT_CH], dt.float32, kind="ExternalInput")
    biasw = nc.dram_tensor("biasw", [OUT_CH, 1], dt.float32, kind="ExternalInput")
    outp = nc.dram_tensor(
        "outp", [OUT_CH, nblocks * BLOCK], dt.float32, kind="ExternalOutput"
    )

    f32r = dt.float32r
    with tile.TileContext(nc) as tc:
        with (
            tc.tile_pool(name="const", bufs=1) as constp,
            tc.tile_pool(name="idx", bufs=3) as idxp,
            tc.tile_pool(name="gath", bufs=8) as gathp,
            tc.tile_pool(name="tsb", bufs=8) as tsbp,
            tc.tile_pool(name="osb", bufs=3) as osbp,
            tc.tile_pool(name="tps", bufs=4, space="PSUM") as tpsp,
            tc.tile_pool(name="ops", bufs=3, space="PSUM") as opsp,
        ):
            wt_f32 = constp.tile([128, 5 * OUT_CH], dt.float32)
            nc.sync.dma_start(out=wt_f32[:], in_=wt[:])
            wt_sb = constp.tile([128, 5 * OUT_CH], f32r)
            nc.vector.tensor_copy(out=wt_sb[:], in_=wt_f32[:])
            bias_sb = constp.tile([OUT_CH, 1], dt.float32)
            nc.sync.dma_start(out=bias_sb[:], in_=biasw[:])
            ident = constp.tile([128, 128], dt.float32)
            make_identity(nc, ident[:])

            idx_sb = None
            for b in range(nblocks):
                bi = b % IDX_BATCH
                if bi == 0:
                    nb = min(IDX_BATCH, nblocks - b)
                    idx_sb = idxp.tile([SUB, IDX_BATCH * JPB], dt.int32)
                    nc.sync.dma_start(
                        out=idx_sb[:, : nb * JPB],
                        in_=adjt[:, b * JPB : (b + nb) * JPB],
                    )
                gath = gathp.tile([SUB, SPB * FW], dt.float32)
                # HW contract: one index per partition per indirect DMA, each
                # gathering one contiguous 64-elem row to its partition.
                for j in range(JPB):
                    nc.gpsimd.indirect_dma_start(
                        out=gath[:].rearrange("p (j c) -> p j c", c=IN_CH)[:, j],
                        out_offset=None,
                        in_=feats[:],
                        in_offset=bass.IndirectOffsetOnAxis(
                            ap=idx_sb[:, bi * JPB + j : bi * JPB + j + 1], axis=0
                        ),
                    )
                out_ps = opsp.tile([OUT_CH, BLOCK], dt.float32, space="PSUM")
                for q in range(5):
                    kq = FCH[q]
                    t_ps = tpsp.tile([128, BLOCK], dt.float32, space="PSUM")
                    for sb in range(SPB):
                        src = gath[:, sb * FW + q * 128 : sb * FW + q * 128 + kq]
                        nc.tensor.transpose(
                            out=t_ps[:kq, sb * SUB : (sb + 1) * SUB],
                            in_=src,
                            identity=ident[:],
                        )
                    t_sb = tsbp.tile([128, BLOCK], f32r)
                    nc.any.tensor_copy(out=t_sb[:kq, :], in_=t_ps[:kq, :])
                    nc.tensor.matmul(
                        out=out_ps[:],
                        lhsT=wt_sb[:kq, q * OUT_CH : (q + 1) * OUT_CH],
                        rhs=t_sb[:kq, :],
                        start=(q == 0),
                        stop=(q == 4),
                    )
                out_sb = osbp.tile([OUT_CH, BLOCK], dt.float32)
                nc.scalar.activation(
                    out_sb[:],
                    out_ps[:],
                    mybir.ActivationFunctionType.Identity,
                    bias=bias_sb[:],
                )
                nc.sync.dma_start(
                    out=outp[:, b * BLOCK : (b + 1) * BLOCK], in_=out_sb[:]
                )
    nc.compile()
    return nc


def _get_nc(nblocks, n_feats, n_cores):
    key = (nblocks, n_feats, n_cores)
    if key not in _NC_CACHE:
        _NC_CACHE[key] = _build(nblocks, n_feats, n_cores)
    return _NC_CACHE[key]


def _prep_adjt(adj, per_core, nblocks):
    """adj [n, KK] int32 -> per-core [SUB, nblocks*JPB] index layout.

    adjt[c][p, b*JPB + sb*KK + k] = adj[c*per_core + b*BLOCK + sb*SUB + p, k]
    (zero padded past the real rows).
    """
    n = adj.shape[0]
    pad = np.zeros((N_CORES, nblocks * BLOCK, KK), np.int32)
    for c in range(N_CORES):
        lo = c * per_core
        cnt = max(0, min(per_core, n - lo))
        if cnt:
            pad[c, :cnt] = adj[lo : lo + cnt]
    # [c, nblocks, SPB, SUB, KK] -> [c, SUB, nblocks, SPB, KK]
    arr = pad.reshape(N_CORES, nblocks, SPB, SUB, KK)
    arr = np.ascontiguousarray(arr.transpose(0, 3, 1, 2, 4))
    return arr.reshape(N_CORES, SUB, nblocks * JPB)


def _prep_wt(W):
    """W [64, 576] -> [128, 5*64] chunked-transposed stationary layout."""
    WT = np.ascontiguousarray(W.T.astype(np.float32))  # [576, 64]
    wt = np.zeros((128, 5 * OUT_CH), np.float32)
    for q in range(5):
        kq = FCH[q]
        wt[:kq, q * OUT_CH : (q + 1) * OUT_CH] = WT[q * 128 : q * 128 + kq]
    return wt


def kernel(in_feats, mask, adj_ids, conv_weight, conv_bias):
    import sys

    if "/opt/trn_rl_repo" not in sys.path:
        sys.path.insert(0, "/opt/trn_rl_repo")
    from concourse.bass_utils import run_bass_kernel_spmd

    in_feats = np.ascontiguousarray(np.asarray(in_feats, dtype=np.float32))
    mask_np = np.asarray(mask).astype(bool)
    adj = np.asarray(adj_ids).astype(np.int32)
    W = np.asarray(conv_weight, dtype=np.float32)
    bvec = np.asarray(conv_bias, dtype=np.float32)

    n = adj.shape[0]
    n_feats = in_feats.shape[0]
    per_core = (n + N_CORES - 1) // N_CORES
    nblocks = (per_core + BLOCK - 1) // BLOCK

    adjt = _prep_adjt(adj, per_core, nblocks)
    wt_np = _prep_wt(W)
    bias_np = np.ascontiguousarray(bvec.reshape(OUT_CH, 1))

    nc = _get_nc(nblocks, n_feats, N_CORES)
    in_maps = [
        {
            "feats": in_feats,
            "adjt": np.ascontiguousarray(adjt[c]),
            "wt": wt_np,
            "biasw": bias_np,
        }
        for c in range(N_CORES)
    ]
    res = run_bass_kernel_spmd(nc, in_maps, core_ids=list(range(N_CORES)))
    globals()["_LAST_RESULTS"] = res

    conv = np.empty((n, OUT_CH), np.float32)
    for c in range(N_CORES):
        lo = c * per_core
        cnt = max(0, min(per_core, n - lo))
        if cnt:
            conv[lo : lo + cnt] = res.results[c]["outp"][:, :cnt].T
    if mask_np.all():
        return conv
    pos = np.cumsum(mask_np.astype(np.int32)) - 1
    pos = np.clip(pos, 0, n - 1)
    return np.where(mask_np[:, None], conv[pos], np.float32(0))

